# revision 21
# baseline (speedup 1.0000x reference)
"""Trainium2 Bass kernel for nn_BSCNUTrain: incremental random-feature network
training (candidate scoring + incremental Gram-Schmidt QR), data-parallel over
the 30000-sample dimension on 8 NeuronCores.

Self-contained: hardcodes shapes/sharding; host reproduces the reference's jax
RNG on CPU, the device runs one fused Bass program for all 12 neuron-addition
iterations (X stays resident in SBUF; one small AllReduce per iteration), and
the host finishes with a tiny triangular solve.
"""

import numpy as np

# ---- problem constants (from the reference) ----
LAMBDAS = (1.0, 10.0)
MAX_NEURONS = 12
RECONFIG = 50
INIT_BATCH = 4000
BATCH_INC = 2166
N_CAND = len(LAMBDAS) * RECONFIG  # 100
N = 30000
D = 784
N_CORES = 8
P = 128  # SBUF partitions


# ---------------------------------------------------------------- host RNG --
def _host_rng(n, d, n_iters):
    """Reproduce the reference's per-iteration randomness exactly (jax on CPU)."""
    import jax
    import jax.numpy as jnp

    cpu = jax.devices("cpu")[0]
    with jax.default_device(cpu):
        # Pin the PRNG impl: this container's boot overrides the default to
        # "rbg", which yields a different stream than the reference's vanilla
        # jax (threefry2x32).
        key = jax.random.key(42, impl="threefry2x32")
        lam = jnp.repeat(jnp.asarray(LAMBDAS, dtype=jnp.float32), RECONFIG)
        iters = []
        batch_size = INIT_BATCH
        for _ in range(n_iters):
            key, kperm, kw, kb = jax.random.split(key, 4)
            indices = jax.random.permutation(kperm, n)[:batch_size]
            w = lam * (2.0 * jax.random.uniform(kw, (d, N_CAND), dtype=jnp.float32) - 1.0)
            b = lam * (2.0 * jax.random.uniform(kb, (1, N_CAND), dtype=jnp.float32) - 1.0)
            iters.append((np.asarray(indices), np.asarray(w), np.asarray(b)))
            batch_size += BATCH_INC
    return iters


# ------------------------------------------------------------ bass program --
def build_program(nrows, d, n_iters, n_cores, n_cand=N_CAND, no_cc=False):
    """Build the full unrolled Bass program.

    nrows: rows per core; d: real feature dim (ones-row appended at index d).
    """
    import concourse.bass as bass
    import concourse.mybir as mybir
    import concourse.tile as tile
    import concourse.bacc as bacc

    f32 = mybir.dt.float32
    u32 = mybir.dt.uint32
    AF = mybir.ActivationFunctionType
    K = n_iters

    ndb = (d + 1 + P - 1) // P          # dblocks of 128 over d+1 (ones row)
    daug = ndb * P
    nblk = (nrows + P - 1) // P         # row blocks per core
    lastb = nrows - (nblk - 1) * P      # rows in last (ragged) block
    SW = 23                             # S columns: Y(10) | hc(1) | Q(12)
    PW = 211                            # payload width: AD(100) | G(11) | den(100)

    nc = bacc.Bacc("TRN2", target_bir_lowering=False, debug=False,
                   num_devices=n_cores)

    xt_d = nc.dram_tensor("XT", [daug, nrows], mybir.dt.float32r, kind="ExternalInput")
    w_d = nc.dram_tensor("WK", [(K // 2) * daug, 256], mybir.dt.float32r, kind="ExternalInput")
    s_d = nc.dram_tensor("SINIT", [P, nblk * SW], f32, kind="ExternalInput")
    m_d = nc.dram_tensor("MASK", [P, K * nblk], f32, kind="ExternalInput")
    bests_d = nc.dram_tensor("BESTS", [K, 8], u32, kind="ExternalOutput")
    r_d = nc.dram_tensor("ROUT", [MAX_NEURONS, MAX_NEURONS], f32, kind="ExternalOutput")
    qt_d = nc.dram_tensor("QTOUT", [MAX_NEURONS, 10], f32, kind="ExternalOutput")
    vs_d = nc.dram_tensor("VSOUT", [K, n_cand], f32, kind="ExternalOutput")

    with tile.TileContext(nc) as tc:
        with tc.tile_pool(name="per", bufs=1) as per, \
             tc.tile_pool(name="wp", bufs=2) as wp, \
             tc.tile_pool(name="mhp", bufs=3) as mhp, \
             tc.tile_pool(name="sml", bufs=2) as sml, \
             tc.tile_pool(name="hps", bufs=2, space="PSUM") as hps, \
             tc.tile_pool(name="accps", bufs=1, space="PSUM") as accps, \
             tc.tile_pool(name="tinyps", bufs=3, space="PSUM") as tinyps, \
             tc.tile_pool(name="dram", bufs=1, space="DRAM") as dram:

            # ---- persistent SBUF state ----
            xt = per.tile([P, ndb, nrows], mybir.dt.float32r)
            nc.sync.dma_start(xt[:], xt_d.ap().rearrange("(db p) r -> p db r", p=P))
            s = per.tile([P, nblk, SW], f32)
            nc.sync.dma_start(s[:], s_d.ap().rearrange("p (b c) -> p b c", c=SW))
            msk = per.tile([P, K, nblk], f32)
            nc.sync.dma_start(msk[:], m_d.ap().rearrange("p (k b) -> p k b", b=nblk))
            # h double-buffer, J=2 iteration groups (group g: iterations
            # 2g at cols 0:100, 2g+1 at cols 100:200).
            assert K % 2 == 0
            h0 = per.tile([P, nblk, 2 * n_cand], f32)
            nc.vector.memset(h0[:], 0.0)
            h1 = per.tile([P, nblk, 2 * n_cand], f32)
            nc.vector.memset(h1[:], 0.0)
            h_tiles = [h0, h1]
            ones = per.tile([P, 1], f32)
            nc.vector.memset(ones[:], 1.0)
            ones_row = per.tile([1, P], f32)
            nc.vector.memset(ones_row[:], 1.0)
            qt_sb = per.tile([MAX_NEURONS, 10], f32)     # QT rows 0..k-2
            qtn_sb = per.tile([MAX_NEURONS, 10], f32)    # -QT
            stage = per.tile([SW, PW], f32)
            nc.vector.memset(stage[:], 0.0)
            zer = per.tile([MAX_NEURONS, MAX_NEURONS], f32)
            nc.vector.memset(zer[:], 0.0)
            nc.sync.dma_start(r_d.ap(), zer[:])

            def blk_rows(b):
                return lastb if b == nblk - 1 else P

            NWIDE = 256   # moving-dim >= 256 so float32r streams 1 cyc/row

            def load_group(g):
                """W for iterations 2g, 2g+1 side by side, zero-padded to 256
                (host pre-interleaves the group layout)."""
                wk = wp.tile([P, ndb, NWIDE], mybir.dt.float32r, tag="wk")
                nc.sync.dma_start(
                    wk[:],
                    w_d.ap()[g * daug:(g + 1) * daug, :]
                    .rearrange("(db p) c -> p db c", p=P),
                )
                return wk

            def phase_a(g, wk, blocks=None):
                """h for group g = sigmoid(X @ [W_2g | W_2g+1 | 0-pad])."""
                hbuf = h_tiles[g % 2]
                for b in (range(nblk) if blocks is None else blocks):
                    bs = blk_rows(b)
                    hp = hps.tile([P, NWIDE], f32, tag="h")
                    for db in range(ndb):
                        nc.tensor.matmul(
                            hp[0:bs, :],
                            xt[:, db, b * P:b * P + bs],
                            wk[:, db, :],
                            start=(db == 0), stop=(db == ndb - 1),
                        )
                    nc.scalar.activation(hbuf[0:bs, b, :], hp[0:bs, 0:2 * n_cand],
                                         AF.Sigmoid)

            # per-iteration work; software-pipelined: the next group's h
            # matmuls run while AllReduces are in flight.
            wk_next = load_group(0)
            phase_a(0, wk_next)
            for k in range(K + 1):
                last_round = k == K
                hbuf = h_tiles[(k // 2) % 2]
                hcol = (k % 2) * n_cand

                nad = 10 + k if k >= 1 else 10   # AD rows: A(10) | C | DQ(k-1)
                nad = min(nad, 10 + MAX_NEURONS)
                ad_ps = den_ps = g_ps = None
                if not last_round:
                    ad_ps = accps.tile([SW, n_cand], f32, tag="ad")
                    den_ps = accps.tile([1, n_cand], f32, tag="den")
                if k >= 1:
                    g_ps = accps.tile([MAX_NEURONS, 11], f32, tag="g")

                for b in range(nblk):
                    if not last_round:
                        mh = mhp.tile([P, n_cand], f32, tag="mh")
                        nc.vector.tensor_scalar_mul(
                            mh[:], hbuf[:, b, hcol:hcol + n_cand], msk[:, k, b:b + 1])
                        mh2 = mhp.tile([P, n_cand], f32, tag="mh2")
                        nc.scalar.activation(mh2[:], mh[:], AF.Square)
                        nc.tensor.matmul(ad_ps[0:nad, :], s[:, b, 0:nad], mh[:],
                                         start=(b == 0), stop=(b == nblk - 1))
                        nc.tensor.matmul(den_ps[:], ones[:], mh2[:],
                                         start=(b == 0), stop=(b == nblk - 1))
                    if k >= 1:
                        nc.tensor.matmul(g_ps[0:k, :], s[:, b, 10:10 + k], s[:, b, 0:11],
                                         start=(b == 0), stop=(b == nblk - 1))

                # ---- stage partials + AllReduce ----
                prow = nad if not last_round else k
                ar_in = dram.tile([prow, PW], f32, tag=f"ari{k}")
                ar_out = dram.tile([prow, PW], f32, tag=f"aro{k}")
                if not last_round:
                    nc.vector.tensor_copy(stage[0:nad, 0:n_cand], ad_ps[0:nad, :])
                    nc.vector.tensor_copy(stage[0:1, 111:211], den_ps[:])
                if k >= 1:
                    nc.vector.tensor_copy(stage[0:k, 100:111], g_ps[0:k, :])
                nc.sync.dma_start(ar_in[:], stage[0:prow, :])
                if no_cc:
                    nc.sync.dma_start(ar_out[:], ar_in[:])
                else:
                    nc.gpsimd.collective_compute(
                        "AllReduce", mybir.AluOpType.add,
                        replica_groups=[list(range(n_cores))],
                        ins=[ar_in.opt()], outs=[ar_out.opt()],
                    )

                # ---- pipeline: compute the next group's h while ARs are
                # in flight; first half on even k, second half on odd k, with
                # the post-AR tiny matmuls in between (strict-FIFO PE queue).
                gn = k // 2 + 1
                if not last_round and 2 * gn < K and k % 2 == 0:
                    wk_next = load_group(gn)
                    phase_a(gn, wk_next, range(nblk // 2))

                # ---- post-AR loads ----
                if not last_round:
                    a_sb = sml.tile([10, n_cand], f32, tag="a")
                    nc.sync.dma_start(a_sb[:], ar_out[0:10, 0:n_cand])
                    den_sb = sml.tile([1, n_cand], f32, tag="densb")
                    nc.sync.dma_start(den_sb[:], ar_out[0:1, 111:211])
                if k >= 1:
                    crow = None
                    if not last_round:
                        crow = sml.tile([1, n_cand], f32, tag="crow")
                        nc.sync.dma_start(crow[:], ar_out[10:11, 0:n_cand])
                        if k >= 2:
                            dq = sml.tile([MAX_NEURONS, n_cand], f32, tag="dq")
                            nc.sync.dma_start(dq[0:k - 1, :], ar_out[11:10 + k, 0:n_cand])
                    hy = sml.tile([1, 10], f32, tag="hy")
                    nc.sync.dma_start(hy[:], ar_out[0:1, 100:110])
                    hh = sml.tile([1, 1], f32, tag="hh")
                    nc.sync.dma_start(hh[:], ar_out[0:1, 110:111])
                    if k >= 2:
                        r1c = sml.tile([MAX_NEURONS, 1], f32, tag="r1c")
                        nc.sync.dma_start(r1c[0:k - 1, :], ar_out[1:k, 110:111])
                    r1r = sml.tile([1, MAX_NEURONS], f32, tag="r1r")
                    if k >= 2:
                        nc.sync.dma_start(r1r[0:1, 0:k - 1], ar_out[1:k, 110:111])

                    # ---- finish GS for column k-1 ----
                    tt = sml.tile([1, 1], f32, tag="tt")
                    if k >= 2:
                        nr1 = tinyps.tile([1, 1], f32, tag="tiny")
                        nc.tensor.matmul(nr1[:], r1c[0:k - 1, :], r1c[0:k - 1, :],
                                         start=True, stop=True)
                        nc.vector.tensor_sub(tt[:], hh[:], nr1[:])
                    else:
                        nc.vector.tensor_copy(tt[:], hh[:])
                    r2 = sml.tile([1, 1], f32, tag="r2")
                    nc.scalar.activation(r2[:], tt[:], AF.Sqrt)
                    # inv_r2 goes into r1r[0, k-1] so r1r[0, 0:k] = [r1 | inv_r2]
                    nc.vector.reciprocal(r1r[0:1, k - 1:k], r2[:])

                    qtn_row = sml.tile([1, 10], f32, tag="qtnrow")
                    if k >= 2:
                        rqt = tinyps.tile([1, 10], f32, tag="tiny")
                        nc.tensor.matmul(rqt[:], r1c[0:k - 1, :], qt_sb[0:k - 1, :],
                                         start=True, stop=True)
                        nc.vector.tensor_sub(qtn_row[:], hy[:], rqt[:])
                        nc.vector.tensor_scalar_mul(qtn_row[:], qtn_row[:],
                                                    r1r[0:1, k - 1:k])
                    else:
                        nc.vector.tensor_scalar_mul(qtn_row[:], hy[:],
                                                    r1r[0:1, k - 1:k])
                    # persist QT row k-1 (+ negated) and R column k-1
                    nc.sync.dma_start(qt_d.ap()[k - 1:k, :], qtn_row[:])
                    nc.sync.dma_start(qt_sb[k - 1:k, :], qtn_row[:])
                    nqt_row = sml.tile([1, 10], f32, tag="nqtrow")
                    nc.vector.tensor_scalar_mul(nqt_row[:], qtn_row[:], -1.0)
                    nc.sync.dma_start(qtn_sb[k - 1:k, :], nqt_row[:])
                    if k >= 2:
                        nc.sync.dma_start(r_d.ap()[0:k - 1, k - 1:k], r1c[0:k - 1, :])
                    nc.sync.dma_start(r_d.ap()[k - 1:k, k - 1:k], r2[:])

                if not last_round:
                    # ---- scoring: num = A - QT^T DQ - qtn_row ⊗ e ----
                    num_ps = tinyps.tile([10, n_cand], f32, tag="tiny")
                    if k >= 1:
                        e_sb = sml.tile([1, n_cand], f32, tag="e")
                        if k >= 2:
                            e_ps = tinyps.tile([1, n_cand], f32, tag="tiny")
                            nc.tensor.matmul(e_ps[:], r1c[0:k - 1, :], dq[0:k - 1, :],
                                             start=True, stop=True)
                            nc.vector.tensor_sub(e_sb[:], crow[:], e_ps[:])
                            nc.vector.tensor_scalar_mul(e_sb[:], e_sb[:],
                                                        r1r[0:1, k - 1:k])
                        else:
                            nc.vector.tensor_scalar_mul(e_sb[:], crow[:],
                                                        r1r[0:1, k - 1:k])
                        if k >= 2:
                            nc.tensor.matmul(num_ps[:], qtn_sb[0:k - 1, :],
                                             dq[0:k - 1, :], start=True, stop=False)
                        nc.tensor.matmul(num_ps[:], nqt_row[:], e_sb[:],
                                         start=(k == 1), stop=True)
                        num_sb = sml.tile([10, n_cand], f32, tag="num")
                        nc.vector.tensor_add(num_sb[:], a_sb[:], num_ps[:])
                    else:
                        num_sb = a_sb
                    sq = sml.tile([10, n_cand], f32, tag="sq")
                    nc.scalar.activation(sq[:], num_sb[:], AF.Square)
                    vs_ps = tinyps.tile([1, n_cand], f32, tag="tiny")
                    nc.tensor.matmul(vs_ps[:], ones[0:10, :], sq[:],
                                     start=True, stop=True)
                    rec = sml.tile([1, n_cand], f32, tag="rec")
                    nc.vector.reciprocal(rec[:], den_sb[:])
                    vs = sml.tile([1, n_cand], f32, tag="vs")
                    nc.vector.tensor_mul(vs[:], vs_ps[:], rec[:])
                    mx = sml.tile([1, 8], f32, tag="mx")
                    mi = sml.tile([1, 8], u32, tag="mi")
                    nc.sync.dma_start(vs_d.ap()[k:k + 1, :], vs[:])
                    nc.vector.max_with_indices(mx[:], mi[:], vs[:])
                    nc.sync.dma_start(bests_d.ap()[k:k + 1, :], mi[:])

                if k >= 1 and not last_round:
                    # ---- broadcast [r1 | inv_r2] to all partitions ----
                    bc_ps = tinyps.tile([P, MAX_NEURONS], f32, tag="tiny")
                    nc.tensor.matmul(bc_ps[:, 0:k], ones_row[:], r1r[0:1, 0:k],
                                     start=True, stop=True)
                    # ---- build q_{k-1} into S col 10+k ----
                    # q = (hc - Q @ r1) * inv_r2 ; hc at S col 10, Q at 11..
                    qcol = sml.tile([P, nblk], f32, tag="qcol")
                    if k >= 2:
                        qtmp = sml.tile([P, nblk, MAX_NEURONS], f32, tag="qtmp")
                        nc.vector.tensor_mul(
                            qtmp[:, :, 0:k - 1],
                            s[:, :, 11:11 + k - 1],
                            bc_ps[:, None, 0:k - 1].broadcast_to([P, nblk, k - 1]),
                        )
                        qr1 = sml.tile([P, nblk, 1], f32, tag="qr1")
                        nc.vector.reduce_sum(qr1[:], qtmp[:, :, 0:k - 1],
                                             axis=mybir.AxisListType.X)
                        nc.vector.tensor_sub(qcol[:], s[:, :, 10], qr1[:, :, 0])
                        nc.vector.tensor_scalar_mul(s[:, :, 10 + k], qcol[:],
                                                    bc_ps[:, k - 1:k])
                    else:
                        nc.vector.tensor_scalar_mul(s[:, :, 11], s[:, :, 10],
                                                    bc_ps[:, 0:1])

                if not last_round:
                    # ---- extract h[:, :, best] -> S[:, :, 10] (pending col) ----
                    with tc.tile_critical():
                        breg = nc.vector.alloc_register(f"best{k}")
                        nc.vector.reg_load(breg, mi[0:1, 0:1])
                        bval = nc.vector.snap(breg, donate=True, min_val=0,
                                              max_val=n_cand - 1)
                        nc.vector.tensor_copy(
                            s[:, :, 10:11],
                            hbuf[:, :, bass.ds(bval + hcol, 1)])

                if not last_round and 2 * gn < K and k % 2 == 1:
                    phase_a(gn, wk_next, range(nblk // 2, nblk))

    nc.compile()
    return nc


# ---------------------------------------------------------------- wrapper --
_CACHE = {}


def _get_program():
    key = (N // N_CORES, D, MAX_NEURONS, N_CORES)
    if key not in _CACHE:
        _CACHE[key] = build_program(*key)
    return _CACHE[key]


def make_inputs(X, Y, rngs, nrows, d, n_iters, n_cores):
    """Build per-core input dicts for the device program."""
    ndb = (d + 1 + P - 1) // P
    daug = ndb * P
    nblk = (nrows + P - 1) // P
    SW = 23
    n = n_cores * nrows

    wk = np.zeros(((n_iters // 2) * daug, 256), dtype=np.float32)
    for k, (_, w, b) in enumerate(rngs):
        g, j = k // 2, (k % 2) * N_CAND
        wk[g * daug:g * daug + d, j:j + N_CAND] = w
        wk[g * daug + d, j:j + N_CAND] = b[0]

    masks = np.zeros((n_iters, n), dtype=np.float32)
    for k, (idx, _, _) in enumerate(rngs):
        masks[k, idx] = 1.0

    in_maps = []
    for c in range(n_cores):
        rows = slice(c * nrows, (c + 1) * nrows)
        xt = np.zeros((daug, nrows), dtype=np.float32)
        xt[:d] = X[rows].T
        xt[d] = 1.0
        sinit = np.zeros((P, nblk, SW), dtype=np.float32)
        m_c = np.zeros((P, n_iters, nblk), dtype=np.float32)
        yc = Y[rows]
        mc = masks[:, rows]
        for b in range(nblk):
            bs = min(P, nrows - b * P)
            sinit[:bs, b, 0:10] = yc[b * P:b * P + bs]
            m_c[:bs, :, b] = mc[:, b * P:b * P + bs].T
        in_maps.append({
            "XT": xt,
            "WK": wk,
            "SINIT": sinit.reshape(P, nblk * SW),
            "MASK": m_c.reshape(P, n_iters * nblk),
        })
    return in_maps


def _solve_outputs(rngs, bests, R, QT, n_iters):
    W = np.stack([rngs[k][1][:, bests[k]] for k in range(n_iters)], axis=1)
    b = np.array([rngs[k][2][0, bests[k]] for k in range(n_iters)],
                 dtype=np.float32)
    Ru = np.triu(R).astype(np.float64)
    qt = QT.astype(np.float64)
    nn = Ru.shape[0]
    beta = np.zeros_like(qt)
    for i in range(nn - 1, -1, -1):
        beta[i] = (qt[i] - Ru[i, i + 1:] @ beta[i + 1:]) / Ru[i, i]
    return (W.astype(np.float32), b.astype(np.float32),
            beta.astype(np.float32))


def kernel(X, Y):
    from concourse.bass_utils import run_bass_kernel_spmd

    X = np.ascontiguousarray(np.asarray(X, dtype=np.float32))
    Y = np.ascontiguousarray(np.asarray(Y, dtype=np.float32))
    nrows = N // N_CORES
    rngs = _host_rng(N, D, MAX_NEURONS)
    nc = _get_program()
    in_maps = make_inputs(X, Y, rngs, nrows, D, MAX_NEURONS, N_CORES)
    res = run_bass_kernel_spmd(nc, in_maps, core_ids=list(range(N_CORES)))
    r0 = res.results[0]
    bests = [int(i) for i in r0["BESTS"][:, 0]]
    return _solve_outputs(rngs, bests, r0["ROUT"], r0["QTOUT"], MAX_NEURONS)


if __name__ == "__main__":
    import reference

    inputs = reference.setup_inputs()
    out = kernel(np.asarray(inputs["X"]), np.asarray(inputs["Y"]))
    for name, arr in zip(("W", "b", "beta"), out):
        print(name, arr.shape, arr.dtype)


# revision 23
# speedup vs baseline: 116.3196x; 116.3196x over previous
"""Trainium2 Bass kernel for nn_BSCNUTrain: incremental random-feature network
training (candidate scoring + incremental Gram-Schmidt QR), data-parallel over
the 30000-sample dimension on 8 NeuronCores.

Self-contained: hardcodes shapes/sharding; host reproduces the reference's jax
RNG on CPU, the device runs one fused Bass program for all 12 neuron-addition
iterations (X stays resident in SBUF; one small AllReduce per iteration), and
the host finishes with a tiny triangular solve.
"""

import numpy as np

# ---- problem constants (from the reference) ----
LAMBDAS = (1.0, 10.0)
MAX_NEURONS = 12
RECONFIG = 50
INIT_BATCH = 4000
BATCH_INC = 2166
N_CAND = len(LAMBDAS) * RECONFIG  # 100
N = 30000
D = 784
N_CORES = 8
P = 128  # SBUF partitions


# ---------------------------------------------------------------- host RNG --
def _host_rng(n, d, n_iters):
    """Reproduce the reference's per-iteration randomness exactly (jax on CPU)."""
    import jax
    import jax.numpy as jnp

    cpu = jax.devices("cpu")[0]
    with jax.default_device(cpu):
        # Pin the PRNG impl: this container's boot overrides the default to
        # "rbg", which yields a different stream than the reference's vanilla
        # jax (threefry2x32).
        key = jax.random.key(42, impl="threefry2x32")
        lam = jnp.repeat(jnp.asarray(LAMBDAS, dtype=jnp.float32), RECONFIG)
        iters = []
        batch_size = INIT_BATCH
        for _ in range(n_iters):
            key, kperm, kw, kb = jax.random.split(key, 4)
            indices = jax.random.permutation(kperm, n)[:batch_size]
            w = lam * (2.0 * jax.random.uniform(kw, (d, N_CAND), dtype=jnp.float32) - 1.0)
            b = lam * (2.0 * jax.random.uniform(kb, (1, N_CAND), dtype=jnp.float32) - 1.0)
            iters.append((np.asarray(indices), np.asarray(w), np.asarray(b)))
            batch_size += BATCH_INC
    return iters


# ------------------------------------------------------------ bass program --
def build_program(nrows, d, n_iters, n_cores, n_cand=N_CAND, no_cc=False):
    """Build the full unrolled Bass program.

    nrows: rows per core; d: real feature dim (ones-row appended at index d).
    """
    import concourse.bass as bass
    import concourse.mybir as mybir
    import concourse.tile as tile
    import concourse.bacc as bacc

    f32 = mybir.dt.float32
    u32 = mybir.dt.uint32
    AF = mybir.ActivationFunctionType
    K = n_iters

    ndb = (d + 1 + P - 1) // P          # dblocks of 128 over d+1 (ones row)
    daug = ndb * P
    nblk = (nrows + P - 1) // P         # row blocks per core
    lastb = nrows - (nblk - 1) * P      # rows in last (ragged) block
    SW = 23                             # S columns: Y(10) | hc(1) | Q(12)
    PW = 211                            # payload width: AD(100) | G(11) | den(100)

    nc = bacc.Bacc("TRN2", target_bir_lowering=False, debug=False,
                   num_devices=n_cores)

    xt_d = nc.dram_tensor("XT", [daug, nrows], mybir.dt.float32r, kind="ExternalInput")
    w_d = nc.dram_tensor("WK", [(K // 2) * daug, 256], mybir.dt.float32r, kind="ExternalInput")
    s_d = nc.dram_tensor("SINIT", [P, nblk * SW], f32, kind="ExternalInput")
    m_d = nc.dram_tensor("MASK", [P, K * nblk], f32, kind="ExternalInput")
    bests_d = nc.dram_tensor("BESTS", [K, 8], u32, kind="ExternalOutput")
    r_d = nc.dram_tensor("ROUT", [MAX_NEURONS, MAX_NEURONS], f32, kind="ExternalOutput")
    qt_d = nc.dram_tensor("QTOUT", [MAX_NEURONS, 10], f32, kind="ExternalOutput")
    vs_d = nc.dram_tensor("VSOUT", [K, n_cand], f32, kind="ExternalOutput")

    with tile.TileContext(nc) as tc:
        with tc.tile_pool(name="per", bufs=1) as per, \
             tc.tile_pool(name="wp", bufs=2) as wp, \
             tc.tile_pool(name="mhp", bufs=3) as mhp, \
             tc.tile_pool(name="sml", bufs=2) as sml, \
             tc.tile_pool(name="hps", bufs=2, space="PSUM") as hps, \
             tc.tile_pool(name="accps", bufs=1, space="PSUM") as accps, \
             tc.tile_pool(name="tinyps", bufs=3, space="PSUM") as tinyps, \
             tc.tile_pool(name="dram", bufs=1, space="DRAM") as dram:

            # ---- persistent SBUF state ----
            xt = per.tile([P, ndb, nrows], mybir.dt.float32r)
            nc.sync.dma_start(xt[:], xt_d.ap().rearrange("(db p) r -> p db r", p=P))
            s = per.tile([P, nblk, SW], f32)
            nc.sync.dma_start(s[:], s_d.ap().rearrange("p (b c) -> p b c", c=SW))
            msk = per.tile([P, K, nblk], f32)
            nc.sync.dma_start(msk[:], m_d.ap().rearrange("p (k b) -> p k b", b=nblk))
            # h double-buffer, J=2 iteration groups (group g: iterations
            # 2g at cols 0:100, 2g+1 at cols 100:200).
            assert K % 2 == 0
            h0 = per.tile([P, nblk, 2 * n_cand], f32)
            nc.vector.memset(h0[:], 0.0)
            h1 = per.tile([P, nblk, 2 * n_cand], f32)
            nc.vector.memset(h1[:], 0.0)
            h_tiles = [h0, h1]
            ones = per.tile([P, 1], f32)
            nc.vector.memset(ones[:], 1.0)
            ones_row = per.tile([1, P], f32)
            nc.vector.memset(ones_row[:], 1.0)
            qt_sb = per.tile([MAX_NEURONS, 10], f32)     # QT rows 0..k-2
            qtn_sb = per.tile([MAX_NEURONS, 10], f32)    # -QT
            stage = per.tile([SW, PW], f32)
            nc.vector.memset(stage[:], 0.0)
            zer = per.tile([MAX_NEURONS, MAX_NEURONS], f32)
            nc.vector.memset(zer[:], 0.0)
            nc.sync.dma_start(r_d.ap(), zer[:])

            def blk_rows(b):
                return lastb if b == nblk - 1 else P

            NWIDE = 256   # moving-dim >= 256 so float32r streams 1 cyc/row

            def load_group(g):
                """W for iterations 2g, 2g+1 side by side, zero-padded to 256
                (host pre-interleaves the group layout)."""
                wk = wp.tile([P, ndb, NWIDE], mybir.dt.float32r, tag="wk")
                nc.sync.dma_start(
                    wk[:],
                    w_d.ap()[g * daug:(g + 1) * daug, :]
                    .rearrange("(db p) c -> p db c", p=P),
                )
                return wk

            def phase_a(g, wk, blocks=None):
                """h for group g = sigmoid(X @ [W_2g | W_2g+1 | 0-pad])."""
                hbuf = h_tiles[g % 2]
                for b in (range(nblk) if blocks is None else blocks):
                    bs = blk_rows(b)
                    hp = hps.tile([P, NWIDE], f32, tag="h")
                    for db in range(ndb):
                        nc.tensor.matmul(
                            hp[0:bs, :],
                            xt[:, db, b * P:b * P + bs],
                            wk[:, db, :],
                            start=(db == 0), stop=(db == ndb - 1),
                        )
                    nc.scalar.activation(hbuf[0:bs, b, :], hp[0:bs, 0:2 * n_cand],
                                         AF.Sigmoid)

            # per-iteration work; software-pipelined: the next group's h
            # matmuls run while AllReduces are in flight.
            wk_next = load_group(0)
            phase_a(0, wk_next)
            for k in range(K + 1):
                last_round = k == K
                hbuf = h_tiles[(k // 2) % 2]
                hcol = (k % 2) * n_cand

                nad = 10 + k if k >= 1 else 10   # AD rows: A(10) | C | DQ(k-1)
                nad = min(nad, 10 + MAX_NEURONS)
                ad_ps = den_ps = g_ps = None
                if not last_round:
                    ad_ps = accps.tile([SW, n_cand], f32, tag="ad")
                    den_ps = accps.tile([1, n_cand], f32, tag="den")
                if k >= 1:
                    g_ps = accps.tile([MAX_NEURONS, 11], f32, tag="g")

                for b in range(nblk):
                    if not last_round:
                        mh = mhp.tile([P, n_cand], f32, tag="mh")
                        nc.vector.tensor_scalar_mul(
                            mh[:], hbuf[:, b, hcol:hcol + n_cand], msk[:, k, b:b + 1])
                        mh2 = mhp.tile([P, n_cand], f32, tag="mh2")
                        nc.scalar.activation(mh2[:], mh[:], AF.Square)
                        nc.tensor.matmul(ad_ps[0:nad, :], s[:, b, 0:nad], mh[:],
                                         start=(b == 0), stop=(b == nblk - 1))
                        nc.tensor.matmul(den_ps[:], ones[:], mh2[:],
                                         start=(b == 0), stop=(b == nblk - 1))
                    if k >= 1:
                        nc.tensor.matmul(g_ps[0:k, :], s[:, b, 10:10 + k], s[:, b, 0:11],
                                         start=(b == 0), stop=(b == nblk - 1))

                # ---- stage partials + AllReduce ----
                prow = nad if not last_round else k
                ar_in = dram.tile([prow, PW], f32, tag=f"ari{k}")
                ar_out = dram.tile([prow, PW], f32, tag=f"aro{k}")
                if not last_round:
                    nc.vector.tensor_copy(stage[0:nad, 0:n_cand], ad_ps[0:nad, :])
                    nc.vector.tensor_copy(stage[0:1, 111:211], den_ps[:])
                if k >= 1:
                    nc.vector.tensor_copy(stage[0:k, 100:111], g_ps[0:k, :])
                nc.sync.dma_start(ar_in[:], stage[0:prow, :])
                if no_cc:
                    nc.sync.dma_start(ar_out[:], ar_in[:])
                else:
                    nc.gpsimd.collective_compute(
                        "AllReduce", mybir.AluOpType.add,
                        replica_groups=[list(range(n_cores))],
                        ins=[ar_in.opt()], outs=[ar_out.opt()],
                    )

                # ---- pipeline: compute the next group's h while ARs are
                # in flight; first half on even k, second half on odd k, with
                # the post-AR tiny matmuls in between (strict-FIFO PE queue).
                gn = k // 2 + 1
                if not last_round and 2 * gn < K and k % 2 == 0:
                    wk_next = load_group(gn)
                    phase_a(gn, wk_next, range(nblk // 2))

                # ---- post-AR loads ----
                if not last_round:
                    a_sb = sml.tile([10, n_cand], f32, tag="a")
                    nc.sync.dma_start(a_sb[:], ar_out[0:10, 0:n_cand])
                    den_sb = sml.tile([1, n_cand], f32, tag="densb")
                    nc.sync.dma_start(den_sb[:], ar_out[0:1, 111:211])
                    rec = sml.tile([1, n_cand], f32, tag="rec")
                    nc.vector.reciprocal(rec[:], den_sb[:])
                if k >= 1:
                    crow = None
                    if not last_round:
                        crow = sml.tile([1, n_cand], f32, tag="crow")
                        nc.sync.dma_start(crow[:], ar_out[10:11, 0:n_cand])
                        if k >= 2:
                            dq = sml.tile([MAX_NEURONS, n_cand], f32, tag="dq")
                            nc.sync.dma_start(dq[0:k - 1, :], ar_out[11:10 + k, 0:n_cand])
                    hy = sml.tile([1, 10], f32, tag="hy")
                    nc.sync.dma_start(hy[:], ar_out[0:1, 100:110])
                    hh = sml.tile([1, 1], f32, tag="hh")
                    nc.sync.dma_start(hh[:], ar_out[0:1, 110:111])
                    if k >= 2:
                        r1c = sml.tile([MAX_NEURONS, 1], f32, tag="r1c")
                        nc.sync.dma_start(r1c[0:k - 1, :], ar_out[1:k, 110:111])
                    r1r = sml.tile([1, MAX_NEURONS], f32, tag="r1r")
                    if k >= 2:
                        nc.sync.dma_start(r1r[0:1, 0:k - 1], ar_out[1:k, 110:111])

                    # ---- finish GS for column k-1 ----
                    # r2 = sqrt(hh - ||r1||^2), fused via ACT bias/scale
                    r2 = sml.tile([1, 1], f32, tag="r2")
                    if k >= 2:
                        nr1 = tinyps.tile([1, 1], f32, tag="tiny")
                        nc.tensor.matmul(nr1[:], r1c[0:k - 1, :], r1c[0:k - 1, :],
                                         start=True, stop=True)
                        nc.scalar.activation(r2[:], nr1[:], AF.Sqrt,
                                             bias=hh[0:1, 0:1], scale=-1.0)
                    else:
                        nc.scalar.activation(r2[:], hh[:], AF.Sqrt)
                    # inv_r2 goes into r1r[0, k-1] so r1r[0, 0:k] = [r1 | inv_r2]
                    nc.vector.reciprocal(r1r[0:1, k - 1:k], r2[:])

                    qtn_row = sml.tile([1, 10], f32, tag="qtnrow")
                    if k >= 2:
                        rqt = tinyps.tile([1, 10], f32, tag="tiny")
                        nc.tensor.matmul(rqt[:], r1c[0:k - 1, :], qt_sb[0:k - 1, :],
                                         start=True, stop=True)
                        nc.vector.tensor_sub(qtn_row[:], hy[:], rqt[:])
                        nc.vector.tensor_scalar_mul(qtn_row[:], qtn_row[:],
                                                    r1r[0:1, k - 1:k])
                    else:
                        nc.vector.tensor_scalar_mul(qtn_row[:], hy[:],
                                                    r1r[0:1, k - 1:k])
                    # persist QT row k-1 (+ negated) and R column k-1
                    nc.sync.dma_start(qt_d.ap()[k - 1:k, :], qtn_row[:])
                    nc.sync.dma_start(qt_sb[k - 1:k, :], qtn_row[:])
                    nqt_row = sml.tile([1, 10], f32, tag="nqtrow")
                    nc.vector.tensor_scalar_mul(nqt_row[:], qtn_row[:], -1.0)
                    nc.sync.dma_start(qtn_sb[k - 1:k, :], nqt_row[:])
                    if k >= 2:
                        nc.sync.dma_start(r_d.ap()[0:k - 1, k - 1:k], r1c[0:k - 1, :])
                    nc.sync.dma_start(r_d.ap()[k - 1:k, k - 1:k], r2[:])

                if not last_round:
                    # ---- scoring: num = A - QT^T DQ - qtn_row ⊗ e ----
                    num_ps = tinyps.tile([10, n_cand], f32, tag="tiny")
                    if k >= 1:
                        e_sb = sml.tile([1, n_cand], f32, tag="e")
                        if k >= 2:
                            e_ps = tinyps.tile([1, n_cand], f32, tag="tiny")
                            nc.tensor.matmul(e_ps[:], r1c[0:k - 1, :], dq[0:k - 1, :],
                                             start=True, stop=True)
                            nc.vector.tensor_sub(e_sb[:], crow[:], e_ps[:])
                            nc.vector.tensor_scalar_mul(e_sb[:], e_sb[:],
                                                        r1r[0:1, k - 1:k])
                        else:
                            nc.vector.tensor_scalar_mul(e_sb[:], crow[:],
                                                        r1r[0:1, k - 1:k])
                        if k >= 2:
                            nc.tensor.matmul(num_ps[:], qtn_sb[0:k - 1, :],
                                             dq[0:k - 1, :], start=True, stop=False)
                        nc.tensor.matmul(num_ps[:], nqt_row[:], e_sb[:],
                                         start=(k == 1), stop=True)
                        num_sb = sml.tile([10, n_cand], f32, tag="num")
                        nc.vector.tensor_add(num_sb[:], a_sb[:], num_ps[:])
                    else:
                        num_sb = a_sb
                    sq = sml.tile([10, n_cand], f32, tag="sq")
                    nc.scalar.activation(sq[:], num_sb[:], AF.Square)
                    vs_ps = tinyps.tile([1, n_cand], f32, tag="tiny")
                    nc.tensor.matmul(vs_ps[:], ones[0:10, :], sq[:],
                                     start=True, stop=True)
                    vs = sml.tile([1, n_cand], f32, tag="vs")
                    nc.vector.tensor_mul(vs[:], vs_ps[:], rec[:])
                    mx = sml.tile([1, 8], f32, tag="mx")
                    mi = sml.tile([1, 8], u32, tag="mi")
                    nc.sync.dma_start(vs_d.ap()[k:k + 1, :], vs[:])
                    nc.vector.max_with_indices(mx[:], mi[:], vs[:])
                    nc.sync.dma_start(bests_d.ap()[k:k + 1, :], mi[:])

                if k >= 1 and not last_round:
                    # ---- broadcast [r1 | inv_r2] to all partitions ----
                    bc_ps = tinyps.tile([P, MAX_NEURONS], f32, tag="tiny")
                    nc.tensor.matmul(bc_ps[:, 0:k], ones_row[:], r1r[0:1, 0:k],
                                     start=True, stop=True)
                    # ---- build q_{k-1} into S col 10+k ----
                    # q = (hc - Q @ r1) * inv_r2 ; hc at S col 10, Q at 11..
                    qcol = sml.tile([P, nblk], f32, tag="qcol")
                    if k >= 2:
                        qtmp = sml.tile([P, nblk, MAX_NEURONS], f32, tag="qtmp")
                        nc.vector.tensor_mul(
                            qtmp[:, :, 0:k - 1],
                            s[:, :, 11:11 + k - 1],
                            bc_ps[:, None, 0:k - 1].broadcast_to([P, nblk, k - 1]),
                        )
                        qr1 = sml.tile([P, nblk, 1], f32, tag="qr1")
                        nc.vector.reduce_sum(qr1[:], qtmp[:, :, 0:k - 1],
                                             axis=mybir.AxisListType.X)
                        nc.vector.tensor_sub(qcol[:], s[:, :, 10], qr1[:, :, 0])
                        nc.vector.tensor_scalar_mul(s[:, :, 10 + k], qcol[:],
                                                    bc_ps[:, k - 1:k])
                    else:
                        nc.vector.tensor_scalar_mul(s[:, :, 11], s[:, :, 10],
                                                    bc_ps[:, 0:1])

                if not last_round:
                    # ---- extract h[:, :, best] -> S[:, :, 10] (pending col) ----
                    with tc.tile_critical():
                        breg = nc.vector.alloc_register(f"best{k}")
                        nc.vector.reg_load(breg, mi[0:1, 0:1])
                        bval = nc.vector.snap(breg, donate=True, min_val=0,
                                              max_val=n_cand - 1)
                        nc.vector.tensor_copy(
                            s[:, :, 10:11],
                            hbuf[:, :, bass.ds(bval + hcol, 1)])

                if not last_round and 2 * gn < K and k % 2 == 1:
                    phase_a(gn, wk_next, range(nblk // 2, nblk))

    nc.compile()
    return nc


# ---------------------------------------------------------------- wrapper --
_CACHE = {}


def _get_program():
    key = (N // N_CORES, D, MAX_NEURONS, N_CORES)
    if key not in _CACHE:
        _CACHE[key] = build_program(*key)
    return _CACHE[key]


def make_inputs(X, Y, rngs, nrows, d, n_iters, n_cores):
    """Build per-core input dicts for the device program."""
    ndb = (d + 1 + P - 1) // P
    daug = ndb * P
    nblk = (nrows + P - 1) // P
    SW = 23
    n = n_cores * nrows

    wk = np.zeros(((n_iters // 2) * daug, 256), dtype=np.float32)
    for k, (_, w, b) in enumerate(rngs):
        g, j = k // 2, (k % 2) * N_CAND
        wk[g * daug:g * daug + d, j:j + N_CAND] = w
        wk[g * daug + d, j:j + N_CAND] = b[0]

    masks = np.zeros((n_iters, n), dtype=np.float32)
    for k, (idx, _, _) in enumerate(rngs):
        masks[k, idx] = 1.0

    in_maps = []
    for c in range(n_cores):
        rows = slice(c * nrows, (c + 1) * nrows)
        xt = np.zeros((daug, nrows), dtype=np.float32)
        xt[:d] = X[rows].T
        xt[d] = 1.0
        sinit = np.zeros((P, nblk, SW), dtype=np.float32)
        m_c = np.zeros((P, n_iters, nblk), dtype=np.float32)
        yc = Y[rows]
        mc = masks[:, rows]
        for b in range(nblk):
            bs = min(P, nrows - b * P)
            sinit[:bs, b, 0:10] = yc[b * P:b * P + bs]
            m_c[:bs, :, b] = mc[:, b * P:b * P + bs].T
        in_maps.append({
            "XT": xt,
            "WK": wk,
            "SINIT": sinit.reshape(P, nblk * SW),
            "MASK": m_c.reshape(P, n_iters * nblk),
        })
    return in_maps


def _solve_outputs(rngs, bests, R, QT, n_iters):
    W = np.stack([rngs[k][1][:, bests[k]] for k in range(n_iters)], axis=1)
    b = np.array([rngs[k][2][0, bests[k]] for k in range(n_iters)],
                 dtype=np.float32)
    Ru = np.triu(R).astype(np.float64)
    qt = QT.astype(np.float64)
    nn = Ru.shape[0]
    beta = np.zeros_like(qt)
    for i in range(nn - 1, -1, -1):
        beta[i] = (qt[i] - Ru[i, i + 1:] @ beta[i + 1:]) / Ru[i, i]
    return (W.astype(np.float32), b.astype(np.float32),
            beta.astype(np.float32))


def kernel(X, Y):
    from concourse.bass_utils import run_bass_kernel_spmd

    X = np.ascontiguousarray(np.asarray(X, dtype=np.float32))
    Y = np.ascontiguousarray(np.asarray(Y, dtype=np.float32))
    nrows = N // N_CORES
    rngs = _host_rng(N, D, MAX_NEURONS)
    nc = _get_program()
    in_maps = make_inputs(X, Y, rngs, nrows, D, MAX_NEURONS, N_CORES)
    res = run_bass_kernel_spmd(nc, in_maps, core_ids=list(range(N_CORES)))
    r0 = res.results[0]
    bests = [int(i) for i in r0["BESTS"][:, 0]]
    return _solve_outputs(rngs, bests, r0["ROUT"], r0["QTOUT"], MAX_NEURONS)


if __name__ == "__main__":
    import reference

    inputs = reference.setup_inputs()
    out = kernel(np.asarray(inputs["X"]), np.asarray(inputs["Y"]))
    for name, arr in zip(("W", "b", "beta"), out):
        print(name, arr.shape, arr.dtype)
